# revision 19
# baseline (speedup 1.0000x reference)
"""Trainium2 Bass kernel for nn_AnemllQATLinearV2 (vq_codebook).

Computes y = x @ W^T + bias where
  W[o,i] = lut[indices[o,i]] * M[o,i],
  M      = (A_dir * g) @ B_dir      (rank-4 modulation),
  A_dir  = |scale_A| / max(||.||_col, eps), B_dir = |scale_B| / max(||.||_row, eps),
  g      = softplus(rank_magnitude) + eps.

Sharding over 8 NeuronCores: 2-way on out_features x 4-way on tokens.
Host prepares transposed fp32r-rounded x^T shards and bf16 index shards;
the device builds M^T with K=4 matmuls, dequantizes the LUT on DVE
(fused affine op when the LUT is affine, 16-entry select otherwise),
and runs the main matmul in fp32r with bias fused into the PSUM drain.
"""

import sys
import types

sys.path.insert(0, "/opt/trn_rl_repo")

import numpy as np
import ml_dtypes


def _install_ntff_hook():
    try:
        import antenv.axon_hooks  # noqa: F401

        return
    except ImportError:
        pass
    try:
        from trn_agent_boot.trn_boot import _ntff_profile_via_ctypes
    except ImportError:
        return
    try:
        hook = _ntff_profile_via_ctypes("/opt/axon/libaxon_pjrt.so")
    except OSError:
        hook = None
    mod = types.ModuleType("antenv.axon_hooks")
    mod._hook = hook
    mod.get_axon_ntff_profile_hook = lambda: mod._hook
    mod.set_axon_ntff_profile_hook = lambda h: setattr(mod, "_hook", h)
    sys.modules["antenv.axon_hooks"] = mod
    import antenv

    antenv.axon_hooks = mod


_install_ntff_hook()

import concourse.bass as bass  # noqa: E402
import concourse.tile as tile  # noqa: E402
from concourse import bacc, mybir, bass_utils  # noqa: E402

# Artifact upload targets an internal bucket this environment can't reach.
bass_utils.upload_artifacts = lambda tmpdir: tmpdir

# Optionally flip walrus's --enable-ldw-opt (dedupes repeated LDWEIGHTS of
# the same stationary tile; bass pins it false).
LDW_OPT = False
TB_INNER = False
_orig_run_command = bass_utils.run_command


def _patched_run_command(argv, **kwargs):
    if LDW_OPT and isinstance(argv, list):
        argv = [
            a.replace("--enable-ldw-opt=false", "--enable-ldw-opt=true")
            if isinstance(a, str) else a
            for a in argv
        ]
    return _orig_run_command(argv, **kwargs)


bass_utils.run_command = _patched_run_command

F32 = mybir.dt.float32
F32R = mybir.dt.float32r
BF16 = mybir.dt.bfloat16

NORM_EPS = 1e-6
MAG_EPS = 1e-6

B, S, IN, OUT, R, L = 4, 2048, 2048, 2048, 4, 16
NTOK = B * S            # 8192 tokens
N_CORES = 8
TO, TT = 2, 4           # out-shard x token-shard grid
NO = OUT // TO          # 1024 out features per core
NT = NTOK // TT         # 2048 tokens per core
KT = IN // 128          # 16 K tiles
TB = NT // 512          # 4 token blocks per core
OI = NO // 128          # 8 out tiles per core

# exposed for test.py
LAST_EXEC_NS = None
LAST_RESULTS = None
PROFILE = False

_PROG_CACHE = {}


def _round_fp32r(a: np.ndarray) -> np.ndarray:
    """Round fp32 to fp32r (11 explicit mantissa bits, RNE), fp32 bit layout."""
    u = np.ascontiguousarray(a, dtype=np.float32).view(np.uint32)
    r = (u + 0x7FF + ((u >> 12) & 1)) & np.uint32(0xFFFFF000)
    return r.view(np.float32)


def _affine_fit(lut32):
    k = np.arange(L, dtype=np.float64)
    A = np.stack([k, np.ones(L)], axis=1)
    coef, *_ = np.linalg.lstsq(A, lut32.astype(np.float64), rcond=None)
    resid = lut32.astype(np.float64) - A @ coef
    affine = bool(np.abs(resid).max() <= 1e-5 * (np.abs(lut32).max() + 1e-30))
    return affine, float(coef[0]), float(coef[1])


def _build_affine(a_c: float, b_c: float, col_split: bool = False):
    """Affine-LUT fast path: W^T fully resident (dequant phase), x^T streamed
    per token block with 3D coalesced DMAs (one dma_start per block)."""
    nc = bacc.Bacc("TRN2", debug=False, target_bir_lowering=False)

    xT_d = nc.dram_tensor("xT", (IN, NT), F32R, kind="ExternalInput").ap()
    idxT_d = nc.dram_tensor("idxT", (IN, NO), BF16, kind="ExternalInput").ap()
    agT_d = nc.dram_tensor("agT", (R, NO), F32R, kind="ExternalInput").ap()
    bdir_d = nc.dram_tensor("bdir", (R, IN), F32R, kind="ExternalInput").ap()
    bias_d = nc.dram_tensor("biasc", (128, OI), F32, kind="ExternalInput").ap()
    yT_d = nc.dram_tensor("yT", (NO, NT), F32, kind="ExternalOutput").ap()

    # partition-major 3D views: [p, k, cols]
    xT_3d = xT_d.rearrange("(k p) t -> p k t", p=128)
    idxT_3d = idxT_d.rearrange("(k p) o -> p k o", p=128)

    with tile.TileContext(nc) as tc:
        with (
            tc.tile_pool(name="small", bufs=1) as small,
            tc.tile_pool(name="wt", bufs=1) as wtp,
            tc.tile_pool(name="idx", bufs=1) as idxp,
            tc.tile_pool(name="xin", bufs=2) as xp,
            tc.tile_pool(name="yout", bufs=4) as yp,
            tc.tile_pool(name="mtps", bufs=2, space="PSUM") as mtps,
            tc.tile_pool(name="yps", bufs=4, space="PSUM") as yps,
        ):
            agT_sb = small.tile([R, NO], F32R)
            bdir_sb = small.tile([R, IN], F32R)
            bias_sb = small.tile([128, OI], F32)
            junk = small.tile([128, 1], F32)
            # One ring, ordered by when each tensor is needed: the tiny
            # warmup/MT inputs, then idx (gates the dequant), then x blocks.
            nc.sync.dma_start(bdir_sb[:], bdir_d[:])
            nc.sync.dma_start(agT_sb[:], agT_d[:])
            nc.sync.dma_start(bias_sb[:], bias_d[:])
            idx_t = idxp.tile([128, KT, NO], BF16)
            nc.sync.dma_start(idx_t[:, :, 0:512], idxT_3d[:, :, 0:512])
            nc.sync.dma_start(idx_t[:, :, 512:NO], idxT_3d[:, :, 512:NO])

            # PE warm-up: a dense block of throwaway matmuls at kernel start
            # trips the HAM activity monitor to K=8/8 (2.4 GHz) before the
            # real matmuls begin; otherwise the first ~half of the kernel
            # runs at 1.2 GHz.
            warm_ps = mtps.tile([128, NO], F32, tag="mt_ps", name="warm_ps")
            for _ in range(28):
                nc.tensor.matmul(
                    warm_ps[:, 0:512], bdir_sb[:, 0:128], bdir_sb[:, 0:512],
                    start=True, stop=True,
                )

            wt_sb = wtp.tile([128, KT, NO], F32R)

            # prefetch x for the first token blocks early; split tb0 along K
            # so its first half lands before the dequant chain finishes
            xts = []
            xt0 = xp.tile([128, KT, 512], F32R, tag="xt", name="xt_0")
            nc.sync.dma_start(xt0[:, 0:KT // 2, :], xT_3d[:, 0:KT // 2, 0:512])
            nc.sync.dma_start(xt0[:, KT // 2:KT, :], xT_3d[:, KT // 2:KT, 0:512])
            xts.append(xt0)
            xt1 = xp.tile([128, KT, 512], F32R, tag="xt", name="xt_1")
            nc.sync.dma_start(xt1[:], xT_3d[:, :, 512:1024])
            xts.append(xt1)

            # ---- Phase A: dequant W^T (full-width tiles to cut DVE op count) ----
            for k in range(KT):
                isl = slice(k * 128, (k + 1) * 128)
                mt_ps = mtps.tile([128, NO], F32, tag="mt_ps", name=f"mt_ps_{k}")
                for oh in range(NO // 512):
                    osl = slice(oh * 512, (oh + 1) * 512)
                    nc.tensor.matmul(
                        mt_ps[:, osl], bdir_sb[:, isl], agT_sb[:, osl],
                        start=True, stop=True,
                    )
                nc.vector.affine_mul_reduce(
                    wt_sb[:, k, :], junk[:], idx_t[:, k, :], mt_ps[:],
                    a_c, b_c,
                )

            # ---- Phase B ----
            if TB_INNER:
                for tb in range(2, TB):
                    xt = xp.tile([128, KT, 512], F32R, tag="xt", name=f"xt_{tb}")
                    nc.sync.dma_start(xt[:], xT_3d[:, :, tb * 512:(tb + 1) * 512])
                    xts.append(xt)
                for oi in range(OI):
                    w_os = slice(oi * 128, (oi + 1) * 128)
                    pys = [
                        yps.tile([128, 512], F32, tag="py", name=f"py_{oi}_{tb}")
                        for tb in range(TB)
                    ]
                    for k in range(KT):
                        for tb in range(TB):
                            nc.tensor.matmul(
                                pys[tb][:], wt_sb[:, k, w_os], xts[tb][:, k, :],
                                start=(k == 0), stop=(k == KT - 1),
                            )
                    for tb in range(TB):
                        yt = yp.tile([128, 512], F32, tag="yt", name=f"yt_{oi}_{tb}")
                        nc.scalar.activation(
                            yt[:], pys[tb][:],
                            mybir.ActivationFunctionType.Identity,
                            bias=bias_sb[:, oi:oi + 1],
                        )
                        nc.sync.dma_start(
                            yT_d[oi * 128:(oi + 1) * 128, tb * 512:(tb + 1) * 512],
                            yt[:],
                        )
            else:
                for tb in range(TB):
                    if tb >= 2:
                        xt = xp.tile([128, KT, 512], F32R, tag="xt", name=f"xt_{tb}")
                        nc.sync.dma_start(xt[:], xT_3d[:, :, tb * 512:(tb + 1) * 512])
                        xts.append(xt)
                    xt = xts[tb]
                    for oi in range(OI):
                        w_os = slice(oi * 128, (oi + 1) * 128)
                        py = yps.tile([128, 512], F32)
                        for k in range(KT):
                            nc.tensor.matmul(
                                py[:], wt_sb[:, k, w_os], xt[:, k, :],
                                start=(k == 0), stop=(k == KT - 1),
                            )
                        yt = yp.tile([128, 512], F32)
                        nc.scalar.activation(
                            yt[:], py[:], mybir.ActivationFunctionType.Identity,
                            bias=bias_sb[:, oi:oi + 1],
                        )
                        nc.sync.dma_start(
                            yT_d[oi * 128:(oi + 1) * 128, tb * 512:(tb + 1) * 512],
                            yt[:],
                        )

    nc.compile()
    return nc


def _build_general(lut32: np.ndarray):
    """Arbitrary-LUT path: 16-entry select dequant, W^T fully resident."""
    nc = bacc.Bacc("TRN2", debug=False, target_bir_lowering=False)

    xT_d = nc.dram_tensor("xT", (IN, NT), F32R, kind="ExternalInput").ap()
    idxT_d = nc.dram_tensor("idxT", (IN, NO), BF16, kind="ExternalInput").ap()
    agT_d = nc.dram_tensor("agT", (R, NO), F32R, kind="ExternalInput").ap()
    bdir_d = nc.dram_tensor("bdir", (R, IN), F32R, kind="ExternalInput").ap()
    bias_d = nc.dram_tensor("biasc", (128, OI), F32, kind="ExternalInput").ap()
    yT_d = nc.dram_tensor("yT", (NO, NT), F32, kind="ExternalOutput").ap()

    with tile.TileContext(nc) as tc:
        with (
            tc.tile_pool(name="small", bufs=1) as small,
            tc.tile_pool(name="wt", bufs=1) as wtp,
            tc.tile_pool(name="idx", bufs=4) as idxp,
            tc.tile_pool(name="xin", bufs=2 * KT) as xp,
            tc.tile_pool(name="q", bufs=3) as qp,
            tc.tile_pool(name="yout", bufs=4) as yp,
            tc.tile_pool(name="mtps", bufs=2, space="PSUM") as mtps,
            tc.tile_pool(name="yps", bufs=4, space="PSUM") as yps,
        ):
            agT_sb = small.tile([R, NO], F32R)
            bdir_sb = small.tile([R, IN], F32R)
            bias_sb = small.tile([128, OI], F32)
            nc.sync.dma_start(agT_sb[:], agT_d[:])
            nc.sync.dma_start(bdir_sb[:], bdir_d[:])
            nc.sync.dma_start(bias_sb[:], bias_d[:])

            wt_sb = wtp.tile([128, KT, NO], F32R)

            for oh in range(NO // 512):
                osl = slice(oh * 512, (oh + 1) * 512)
                for k in range(KT):
                    isl = slice(k * 128, (k + 1) * 128)
                    idx_t = idxp.tile([128, 512], BF16)
                    nc.sync.dma_start(idx_t[:], idxT_d[isl, osl])
                    mt_ps = mtps.tile([128, 512], F32)
                    nc.tensor.matmul(
                        mt_ps[:], bdir_sb[:, isl], agT_sb[:, osl],
                        start=True, stop=True,
                    )
                    q_t = qp.tile([128, 512], F32)
                    tmp_t = qp.tile([128, 512], F32)
                    nc.vector.tensor_scalar(
                        q_t[:], idx_t[:], 0.0, float(lut32[0]),
                        mybir.AluOpType.is_equal, mybir.AluOpType.mult,
                    )
                    for l in range(1, L):
                        nc.vector.tensor_scalar(
                            tmp_t[:], idx_t[:], float(l), float(lut32[l]),
                            mybir.AluOpType.is_equal, mybir.AluOpType.mult,
                        )
                        nc.vector.tensor_add(q_t[:], q_t[:], tmp_t[:])
                    nc.vector.tensor_mul(wt_sb[:, k, osl], q_t[:], mt_ps[:])

            for tb in range(TB):
                tsl = slice(tb * 512, (tb + 1) * 512)
                xts = []
                for k in range(KT):
                    xt = xp.tile([128, 512], F32R)
                    nc.sync.dma_start(xt[:], xT_d[k * 128:(k + 1) * 128, tsl])
                    xts.append(xt)
                for oi in range(OI):
                    py = yps.tile([128, 512], F32)
                    for k in range(KT):
                        nc.tensor.matmul(
                            py[:], wt_sb[:, k, oi * 128:(oi + 1) * 128], xts[k][:],
                            start=(k == 0), stop=(k == KT - 1),
                        )
                    yt = yp.tile([128, 512], F32)
                    nc.scalar.activation(
                        yt[:], py[:], mybir.ActivationFunctionType.Identity,
                        bias=bias_sb[:, oi:oi + 1],
                    )
                    nc.sync.dma_start(yT_d[oi * 128:(oi + 1) * 128, tsl], yt[:])

    nc.compile()
    return nc


def kernel(x, indices, lut, scale_A, scale_B, rank_magnitude, bias):
    global LAST_EXEC_NS, LAST_RESULTS

    x = np.asarray(x)
    indices = np.asarray(indices)
    lut32 = np.asarray(lut, dtype=np.float32)
    scale_A = np.asarray(scale_A, dtype=np.float64)
    scale_B = np.asarray(scale_B, dtype=np.float64)
    rank_magnitude = np.asarray(rank_magnitude, dtype=np.float64)
    bias = np.asarray(bias, dtype=np.float32)

    # ---- host: normalized scales (tiny tensors) ----
    A = np.abs(scale_A)                                   # [OUT, R]
    A_dir = A / np.maximum(np.linalg.norm(A, axis=0, keepdims=True), NORM_EPS)
    Bm = np.abs(scale_B)                                  # [R, IN]
    B_dir = Bm / np.maximum(np.linalg.norm(Bm, axis=1, keepdims=True), NORM_EPS)
    g = np.log1p(np.exp(rank_magnitude)) + MAG_EPS        # softplus, [R]
    AgT = (A_dir * g[None, :]).T.astype(np.float32)       # [R, OUT]
    B_dir = B_dir.astype(np.float32)                      # [R, IN]

    affine, a_c, b_c = _affine_fit(lut32)

    cache_key = (affine, LDW_OPT, TB_INNER, lut32.tobytes())
    if cache_key not in _PROG_CACHE:
        _PROG_CACHE[cache_key] = (
            _build_affine(a_c, b_c) if affine else _build_general(lut32)
        )
    nc = _PROG_CACHE[cache_key]

    # ---- host: shard + transpose + round ----
    xT = _round_fp32r(x.reshape(NTOK, IN).T)              # [IN, NTOK]
    idxT = np.ascontiguousarray(indices.T).astype(ml_dtypes.bfloat16)  # [IN, OUT]
    agT_r = _round_fp32r(AgT)
    bdir_r = _round_fp32r(B_dir)

    in_maps = []
    for c in range(N_CORES):
        oc, tc_ = c // TT, c % TT
        in_maps.append({
            "xT": np.ascontiguousarray(xT[:, tc_ * NT:(tc_ + 1) * NT]),
            "idxT": np.ascontiguousarray(idxT[:, oc * NO:(oc + 1) * NO]),
            "agT": np.ascontiguousarray(agT_r[:, oc * NO:(oc + 1) * NO]),
            "bdir": bdir_r,
            "biasc": np.ascontiguousarray(
                bias[oc * NO:(oc + 1) * NO].reshape(OI, 128).T
            ),
        })

    res = bass_utils.run_bass_kernel_spmd(
        nc, in_maps, core_ids=list(range(N_CORES)), trace=PROFILE
    )
    LAST_EXEC_NS = res.exec_time_ns
    LAST_RESULTS = res

    # ---- host: gather ----
    y = np.empty((NTOK, OUT), dtype=np.float32)
    for c in range(N_CORES):
        oc, tc_ = c // TT, c % TT
        yT_c = res.results[c]["yT"]                       # [NO, NT]
        y[tc_ * NT:(tc_ + 1) * NT, oc * NO:(oc + 1) * NO] = yT_c.T
    return y.reshape(B, S, OUT)


# revision 20
# speedup vs baseline: 1.0114x; 1.0114x over previous
"""Trainium2 Bass kernel for nn_AnemllQATLinearV2 (vq_codebook).

Computes y = x @ W^T + bias where
  W[o,i] = lut[indices[o,i]] * M[o,i],
  M      = (A_dir * g) @ B_dir      (rank-4 modulation),
  A_dir  = |scale_A| / max(||.||_col, eps), B_dir = |scale_B| / max(||.||_row, eps),
  g      = softplus(rank_magnitude) + eps.

Sharding over 8 NeuronCores: 2-way on out_features x 4-way on tokens.
Host prepares transposed fp32r-rounded x^T shards and bf16 index shards;
the device builds M^T with K=4 matmuls, dequantizes the LUT on DVE
(fused affine op when the LUT is affine, 16-entry select otherwise),
and runs the main matmul in fp32r with bias fused into the PSUM drain.
"""

import sys
import types

sys.path.insert(0, "/opt/trn_rl_repo")

import numpy as np
import ml_dtypes


def _install_ntff_hook():
    try:
        import antenv.axon_hooks  # noqa: F401

        return
    except ImportError:
        pass
    try:
        from trn_agent_boot.trn_boot import _ntff_profile_via_ctypes
    except ImportError:
        return
    try:
        hook = _ntff_profile_via_ctypes("/opt/axon/libaxon_pjrt.so")
    except OSError:
        hook = None
    mod = types.ModuleType("antenv.axon_hooks")
    mod._hook = hook
    mod.get_axon_ntff_profile_hook = lambda: mod._hook
    mod.set_axon_ntff_profile_hook = lambda h: setattr(mod, "_hook", h)
    sys.modules["antenv.axon_hooks"] = mod
    import antenv

    antenv.axon_hooks = mod


_install_ntff_hook()

import concourse.bass as bass  # noqa: E402
import concourse.tile as tile  # noqa: E402
from concourse import bacc, mybir, bass_utils  # noqa: E402

# Artifact upload targets an internal bucket this environment can't reach.
bass_utils.upload_artifacts = lambda tmpdir: tmpdir

# Optionally flip walrus's --enable-ldw-opt (dedupes repeated LDWEIGHTS of
# the same stationary tile; bass pins it false).
LDW_OPT = False
TB_INNER = False
_orig_run_command = bass_utils.run_command


def _patched_run_command(argv, **kwargs):
    if LDW_OPT and isinstance(argv, list):
        argv = [
            a.replace("--enable-ldw-opt=false", "--enable-ldw-opt=true")
            if isinstance(a, str) else a
            for a in argv
        ]
    return _orig_run_command(argv, **kwargs)


bass_utils.run_command = _patched_run_command

F32 = mybir.dt.float32
F32R = mybir.dt.float32r
BF16 = mybir.dt.bfloat16

NORM_EPS = 1e-6
MAG_EPS = 1e-6

B, S, IN, OUT, R, L = 4, 2048, 2048, 2048, 4, 16
NTOK = B * S            # 8192 tokens
N_CORES = 8
TO, TT = 2, 4           # out-shard x token-shard grid
NO = OUT // TO          # 1024 out features per core
NT = NTOK // TT         # 2048 tokens per core
KT = IN // 128          # 16 K tiles
TB = NT // 512          # 4 token blocks per core
OI = NO // 128          # 8 out tiles per core

# exposed for test.py
LAST_EXEC_NS = None
LAST_RESULTS = None
PROFILE = False

_PROG_CACHE = {}


def _round_fp32r(a: np.ndarray) -> np.ndarray:
    """Round fp32 to fp32r (11 explicit mantissa bits, RNE), fp32 bit layout."""
    u = np.ascontiguousarray(a, dtype=np.float32).view(np.uint32)
    r = (u + 0x7FF + ((u >> 12) & 1)) & np.uint32(0xFFFFF000)
    return r.view(np.float32)


def _affine_fit(lut32):
    k = np.arange(L, dtype=np.float64)
    A = np.stack([k, np.ones(L)], axis=1)
    coef, *_ = np.linalg.lstsq(A, lut32.astype(np.float64), rcond=None)
    resid = lut32.astype(np.float64) - A @ coef
    affine = bool(np.abs(resid).max() <= 1e-5 * (np.abs(lut32).max() + 1e-30))
    return affine, float(coef[0]), float(coef[1])


def _build_affine(a_c: float, b_c: float, col_split: bool = False):
    """Affine-LUT fast path: W^T fully resident (dequant phase), x^T streamed
    per token block with 3D coalesced DMAs (one dma_start per block)."""
    nc = bacc.Bacc("TRN2", debug=False, target_bir_lowering=False)

    xT_d = nc.dram_tensor("xT", (IN, NT), F32R, kind="ExternalInput").ap()
    idxT_d = nc.dram_tensor("idxT", (IN, NO), BF16, kind="ExternalInput").ap()
    agT_d = nc.dram_tensor("agT", (R, NO), F32R, kind="ExternalInput").ap()
    bdir_d = nc.dram_tensor("bdir", (R, IN), F32R, kind="ExternalInput").ap()
    bias_d = nc.dram_tensor("biasc", (128, OI), F32, kind="ExternalInput").ap()
    yT_d = nc.dram_tensor("yT", (NO, NT), F32, kind="ExternalOutput").ap()

    # partition-major 3D views: [p, k, cols]
    xT_3d = xT_d.rearrange("(k p) t -> p k t", p=128)
    idxT_3d = idxT_d.rearrange("(k p) o -> p k o", p=128)

    with tile.TileContext(nc) as tc:
        with (
            tc.tile_pool(name="small", bufs=1) as small,
            tc.tile_pool(name="wt", bufs=1) as wtp,
            tc.tile_pool(name="idx", bufs=1) as idxp,
            tc.tile_pool(name="xin", bufs=2) as xp,
            tc.tile_pool(name="yout", bufs=4) as yp,
            tc.tile_pool(name="mtps", bufs=2, space="PSUM") as mtps,
            tc.tile_pool(name="yps", bufs=4, space="PSUM") as yps,
        ):
            agT_sb = small.tile([R, NO], F32R)
            bdir_sb = small.tile([R, IN], F32R)
            bias_sb = small.tile([128, OI], F32)
            junk = small.tile([128, 1], F32)
            # One ring, ordered by when each tensor is needed: the tiny
            # warmup/MT inputs, then idx (gates the dequant), then x blocks.
            nc.sync.dma_start(bdir_sb[:], bdir_d[:])
            nc.sync.dma_start(agT_sb[:], agT_d[:])
            nc.sync.dma_start(bias_sb[:], bias_d[:])
            idx_t = idxp.tile([128, KT, NO], BF16)
            nc.sync.dma_start(idx_t[:, :, 0:512], idxT_3d[:, :, 0:512])
            nc.sync.dma_start(idx_t[:, :, 512:NO], idxT_3d[:, :, 512:NO])

            # PE warm-up: a dense block of throwaway matmuls at kernel start
            # trips the HAM activity monitor to K=8/8 (2.4 GHz) before the
            # real matmuls begin; otherwise the first ~half of the kernel
            # runs at 1.2 GHz.
            warm_ps = mtps.tile([128, NO], F32, tag="mt_ps", name="warm_ps")
            for _ in range(8):
                nc.tensor.matmul(
                    warm_ps[:, 0:512], bdir_sb[:, 0:128], bdir_sb[:, 0:512],
                    start=True, stop=True,
                )

            wt_sb = wtp.tile([128, KT, NO], F32R)

            # prefetch x for the first token blocks early; split tb0 along K
            # so its first half lands before the dequant chain finishes
            xts = []
            xt0 = xp.tile([128, KT, 512], F32R, tag="xt", name="xt_0")
            nc.sync.dma_start(xt0[:, 0:KT // 2, :], xT_3d[:, 0:KT // 2, 0:512])
            nc.sync.dma_start(xt0[:, KT // 2:KT, :], xT_3d[:, KT // 2:KT, 0:512])
            xts.append(xt0)
            xt1 = xp.tile([128, KT, 512], F32R, tag="xt", name="xt_1")
            nc.sync.dma_start(xt1[:], xT_3d[:, :, 512:1024])
            xts.append(xt1)

            # ---- Phase A: dequant W^T (full-width tiles to cut DVE op count) ----
            for k in range(KT):
                isl = slice(k * 128, (k + 1) * 128)
                mt_ps = mtps.tile([128, NO], F32, tag="mt_ps", name=f"mt_ps_{k}")
                for oh in range(NO // 512):
                    osl = slice(oh * 512, (oh + 1) * 512)
                    nc.tensor.matmul(
                        mt_ps[:, osl], bdir_sb[:, isl], agT_sb[:, osl],
                        start=True, stop=True,
                    )
                nc.vector.affine_mul_reduce(
                    wt_sb[:, k, :], junk[:], idx_t[:, k, :], mt_ps[:],
                    a_c, b_c,
                )

            # ---- Phase B ----
            if TB_INNER:
                for tb in range(2, TB):
                    xt = xp.tile([128, KT, 512], F32R, tag="xt", name=f"xt_{tb}")
                    nc.sync.dma_start(xt[:], xT_3d[:, :, tb * 512:(tb + 1) * 512])
                    xts.append(xt)
                for oi in range(OI):
                    w_os = slice(oi * 128, (oi + 1) * 128)
                    pys = [
                        yps.tile([128, 512], F32, tag="py", name=f"py_{oi}_{tb}")
                        for tb in range(TB)
                    ]
                    for k in range(KT):
                        for tb in range(TB):
                            nc.tensor.matmul(
                                pys[tb][:], wt_sb[:, k, w_os], xts[tb][:, k, :],
                                start=(k == 0), stop=(k == KT - 1),
                            )
                    for tb in range(TB):
                        yt = yp.tile([128, 512], F32, tag="yt", name=f"yt_{oi}_{tb}")
                        nc.scalar.activation(
                            yt[:], pys[tb][:],
                            mybir.ActivationFunctionType.Identity,
                            bias=bias_sb[:, oi:oi + 1],
                        )
                        nc.sync.dma_start(
                            yT_d[oi * 128:(oi + 1) * 128, tb * 512:(tb + 1) * 512],
                            yt[:],
                        )
            else:
                for tb in range(TB):
                    if tb >= 2:
                        xt = xp.tile([128, KT, 512], F32R, tag="xt", name=f"xt_{tb}")
                        nc.sync.dma_start(xt[:], xT_3d[:, :, tb * 512:(tb + 1) * 512])
                        xts.append(xt)
                    xt = xts[tb]
                    for oi in range(OI):
                        w_os = slice(oi * 128, (oi + 1) * 128)
                        py = yps.tile([128, 512], F32)
                        for k in range(KT):
                            nc.tensor.matmul(
                                py[:], wt_sb[:, k, w_os], xt[:, k, :],
                                start=(k == 0), stop=(k == KT - 1),
                            )
                        yt = yp.tile([128, 512], F32)
                        nc.scalar.activation(
                            yt[:], py[:], mybir.ActivationFunctionType.Identity,
                            bias=bias_sb[:, oi:oi + 1],
                        )
                        nc.sync.dma_start(
                            yT_d[oi * 128:(oi + 1) * 128, tb * 512:(tb + 1) * 512],
                            yt[:],
                        )

    nc.compile()
    return nc


def _build_general(lut32: np.ndarray):
    """Arbitrary-LUT path: 16-entry select dequant, W^T fully resident."""
    nc = bacc.Bacc("TRN2", debug=False, target_bir_lowering=False)

    xT_d = nc.dram_tensor("xT", (IN, NT), F32R, kind="ExternalInput").ap()
    idxT_d = nc.dram_tensor("idxT", (IN, NO), BF16, kind="ExternalInput").ap()
    agT_d = nc.dram_tensor("agT", (R, NO), F32R, kind="ExternalInput").ap()
    bdir_d = nc.dram_tensor("bdir", (R, IN), F32R, kind="ExternalInput").ap()
    bias_d = nc.dram_tensor("biasc", (128, OI), F32, kind="ExternalInput").ap()
    yT_d = nc.dram_tensor("yT", (NO, NT), F32, kind="ExternalOutput").ap()

    with tile.TileContext(nc) as tc:
        with (
            tc.tile_pool(name="small", bufs=1) as small,
            tc.tile_pool(name="wt", bufs=1) as wtp,
            tc.tile_pool(name="idx", bufs=4) as idxp,
            tc.tile_pool(name="xin", bufs=2 * KT) as xp,
            tc.tile_pool(name="q", bufs=3) as qp,
            tc.tile_pool(name="yout", bufs=4) as yp,
            tc.tile_pool(name="mtps", bufs=2, space="PSUM") as mtps,
            tc.tile_pool(name="yps", bufs=4, space="PSUM") as yps,
        ):
            agT_sb = small.tile([R, NO], F32R)
            bdir_sb = small.tile([R, IN], F32R)
            bias_sb = small.tile([128, OI], F32)
            nc.sync.dma_start(agT_sb[:], agT_d[:])
            nc.sync.dma_start(bdir_sb[:], bdir_d[:])
            nc.sync.dma_start(bias_sb[:], bias_d[:])

            wt_sb = wtp.tile([128, KT, NO], F32R)

            for oh in range(NO // 512):
                osl = slice(oh * 512, (oh + 1) * 512)
                for k in range(KT):
                    isl = slice(k * 128, (k + 1) * 128)
                    idx_t = idxp.tile([128, 512], BF16)
                    nc.sync.dma_start(idx_t[:], idxT_d[isl, osl])
                    mt_ps = mtps.tile([128, 512], F32)
                    nc.tensor.matmul(
                        mt_ps[:], bdir_sb[:, isl], agT_sb[:, osl],
                        start=True, stop=True,
                    )
                    q_t = qp.tile([128, 512], F32)
                    tmp_t = qp.tile([128, 512], F32)
                    nc.vector.tensor_scalar(
                        q_t[:], idx_t[:], 0.0, float(lut32[0]),
                        mybir.AluOpType.is_equal, mybir.AluOpType.mult,
                    )
                    for l in range(1, L):
                        nc.vector.tensor_scalar(
                            tmp_t[:], idx_t[:], float(l), float(lut32[l]),
                            mybir.AluOpType.is_equal, mybir.AluOpType.mult,
                        )
                        nc.vector.tensor_add(q_t[:], q_t[:], tmp_t[:])
                    nc.vector.tensor_mul(wt_sb[:, k, osl], q_t[:], mt_ps[:])

            for tb in range(TB):
                tsl = slice(tb * 512, (tb + 1) * 512)
                xts = []
                for k in range(KT):
                    xt = xp.tile([128, 512], F32R)
                    nc.sync.dma_start(xt[:], xT_d[k * 128:(k + 1) * 128, tsl])
                    xts.append(xt)
                for oi in range(OI):
                    py = yps.tile([128, 512], F32)
                    for k in range(KT):
                        nc.tensor.matmul(
                            py[:], wt_sb[:, k, oi * 128:(oi + 1) * 128], xts[k][:],
                            start=(k == 0), stop=(k == KT - 1),
                        )
                    yt = yp.tile([128, 512], F32)
                    nc.scalar.activation(
                        yt[:], py[:], mybir.ActivationFunctionType.Identity,
                        bias=bias_sb[:, oi:oi + 1],
                    )
                    nc.sync.dma_start(yT_d[oi * 128:(oi + 1) * 128, tsl], yt[:])

    nc.compile()
    return nc


def kernel(x, indices, lut, scale_A, scale_B, rank_magnitude, bias):
    global LAST_EXEC_NS, LAST_RESULTS

    x = np.asarray(x)
    indices = np.asarray(indices)
    lut32 = np.asarray(lut, dtype=np.float32)
    scale_A = np.asarray(scale_A, dtype=np.float64)
    scale_B = np.asarray(scale_B, dtype=np.float64)
    rank_magnitude = np.asarray(rank_magnitude, dtype=np.float64)
    bias = np.asarray(bias, dtype=np.float32)

    # ---- host: normalized scales (tiny tensors) ----
    A = np.abs(scale_A)                                   # [OUT, R]
    A_dir = A / np.maximum(np.linalg.norm(A, axis=0, keepdims=True), NORM_EPS)
    Bm = np.abs(scale_B)                                  # [R, IN]
    B_dir = Bm / np.maximum(np.linalg.norm(Bm, axis=1, keepdims=True), NORM_EPS)
    g = np.log1p(np.exp(rank_magnitude)) + MAG_EPS        # softplus, [R]
    AgT = (A_dir * g[None, :]).T.astype(np.float32)       # [R, OUT]
    B_dir = B_dir.astype(np.float32)                      # [R, IN]

    affine, a_c, b_c = _affine_fit(lut32)

    cache_key = (affine, LDW_OPT, TB_INNER, lut32.tobytes())
    if cache_key not in _PROG_CACHE:
        _PROG_CACHE[cache_key] = (
            _build_affine(a_c, b_c) if affine else _build_general(lut32)
        )
    nc = _PROG_CACHE[cache_key]

    # ---- host: shard + transpose + round ----
    xT = _round_fp32r(x.reshape(NTOK, IN).T)              # [IN, NTOK]
    idxT = np.ascontiguousarray(indices.T).astype(ml_dtypes.bfloat16)  # [IN, OUT]
    agT_r = _round_fp32r(AgT)
    bdir_r = _round_fp32r(B_dir)

    in_maps = []
    for c in range(N_CORES):
        oc, tc_ = c // TT, c % TT
        in_maps.append({
            "xT": np.ascontiguousarray(xT[:, tc_ * NT:(tc_ + 1) * NT]),
            "idxT": np.ascontiguousarray(idxT[:, oc * NO:(oc + 1) * NO]),
            "agT": np.ascontiguousarray(agT_r[:, oc * NO:(oc + 1) * NO]),
            "bdir": bdir_r,
            "biasc": np.ascontiguousarray(
                bias[oc * NO:(oc + 1) * NO].reshape(OI, 128).T
            ),
        })

    res = bass_utils.run_bass_kernel_spmd(
        nc, in_maps, core_ids=list(range(N_CORES)), trace=PROFILE
    )
    LAST_EXEC_NS = res.exec_time_ns
    LAST_RESULTS = res

    # ---- host: gather ----
    y = np.empty((NTOK, OUT), dtype=np.float32)
    for c in range(N_CORES):
        oc, tc_ = c // TT, c % TT
        yT_c = res.results[c]["yT"]                       # [NO, NT]
        y[tc_ * NT:(tc_ + 1) * NT, oc * NO:(oc + 1) * NO] = yT_c.T
    return y.reshape(B, S, OUT)
